# revision 29
# baseline (speedup 1.0000x reference)
"""Distributed Trainium2 kernel for a dense transformer block.

Reference computation (per batch):
  x = x + o_proj(attn(rope(qkv(rmsnorm(x))), causal)) ; x = x + w2(silu(wg(rmsnorm(x))) * w1(rmsnorm(x)))

Sharding: DP=2 on batch x TP=4 on heads / MLP rows (Megatron).
Cores 0-3 handle batch 0, cores 4-7 batch 1. Within a group, rank r owns
heads 4r..4r+3 and MLP rows 1024r..1024(r+1). Two bf16 AllReduces per
group, each chunked into NQ pieces and software-pipelined against compute.
"""

import sys

sys.path.insert(0, "/opt/trn_rl_repo")

import numpy as np
import ml_dtypes

import concourse.bass as bass
import concourse.bacc as bacc
import concourse.mybir as mybir
import concourse.tile as tile
from concourse.bass_utils import run_bass_kernel_spmd

BF = ml_dtypes.bfloat16
F32 = mybir.dt.float32
BF16 = mybir.dt.bfloat16

D = 1024
NH = 16
DH = 64
MULT = 4
EPS = 1e-5
ROPE_BASE = 10000.0
B = 2
TP = 4  # tensor-parallel ranks per group
HPC = NH // TP  # heads per core = 4
QKF = 2 * HPC * DH  # q+k shard features = 512
VF = HPC * DH  # v shard features = 256
MID = MULT * D // TP  # mlp rows per core = 1024
AF = mybir.ActivationFunctionType
ALU = mybir.AluOpType


def build_nc(T, use_silu=False):
    """Build the SPMD graph for one core (token count T per batch)."""
    DC = D // 128  # d chunks = 8
    TT = T // 128  # token tiles
    QT = min(512, T)  # q-tile width == AR chunk width
    NQ = T // QT
    CPQ = QT // 128  # 128-token tiles per chunk
    MIDC = MID // 128  # mlp row chunks = 8
    NT = D // 512

    nc = bacc.Bacc("TRN2", target_bir_lowering=False, debug=False, num_devices=8)

    x_e = nc.dram_tensor("x", [T, D], F32, kind="ExternalInput")
    qkw_e = nc.dram_tensor("qkw_t", [D, QKF], BF16, kind="ExternalInput")
    vw_e = nc.dram_tensor("vw_m", [D, VF], BF16, kind="ExternalInput")
    ow_e = nc.dram_tensor("ow_m", [VF, D], BF16, kind="ExternalInput")
    w1w_e = nc.dram_tensor("w1w_t", [D, MID], BF16, kind="ExternalInput")
    wgw_e = nc.dram_tensor("wgw_t", [D, MID], BF16, kind="ExternalInput")
    w2w_e = nc.dram_tensor("w2w_m", [MID, D], BF16, kind="ExternalInput")
    cos_e = nc.dram_tensor("cosr", [128, T], BF16, kind="ExternalInput")
    sin_e = nc.dram_tensor("sinr", [128, T], BF16, kind="ExternalInput")
    cm_e = nc.dram_tensor("cmask", [CPQ * 128, QT], BF16, kind="ExternalInput")
    id_e = nc.dram_tensor("ident", [128, 128], BF16, kind="ExternalInput")
    out_e = nc.dram_tensor("out", [T, D], F32, kind="ExternalOutput")

    groups = [[0, 1, 2, 3], [4, 5, 6, 7]]

    with tile.TileContext(nc) as tc:
        with (
            tc.tile_pool(name="const", bufs=1) as cpool,
            tc.tile_pool(name="actfm", bufs=1) as fmpool,
            tc.tile_pool(name="qko", bufs=1) as qkpool,
            tc.tile_pool(name="vaug", bufs=1) as vpool,
            tc.tile_pool(name="xin", bufs=2) as xpool,
            tc.tile_pool(name="xnb", bufs=3) as xnpool,
            tc.tile_pool(name="work", bufs=4) as wpool,
            tc.tile_pool(name="rope", bufs=3) as rpool,
            tc.tile_pool(name="stats", bufs=8) as spool,
            tc.tile_pool(name="psA", bufs=4, space="PSUM") as psA,
            tc.tile_pool(name="psO", bufs=2, space="PSUM") as psO,
            tc.tile_pool(name="psS", bufs=2, space="PSUM") as psS,
            tc.tile_pool(name="dram", bufs=1, space="DRAM") as dpool,
        ):
            # ---- resident weights / tables ----
            def load_tiles(src, width, n, dt=BF16):
                ts = []
                for i in range(n):
                    t = cpool.tile(
                        [128, width], dt, tag=f"{src.name}_{i}", name=f"{src.name}_{i}"
                    )
                    nc.sync.dma_start(t[:], src[i * 128 : (i + 1) * 128, :])
                    ts.append(t)
                return ts

            qkw = load_tiles(qkw_e, QKF, DC)
            vw = load_tiles(vw_e, VF, DC)
            ow = load_tiles(ow_e, D, VF // 128)
            w1r = w1w_e.rearrange("(c p) m -> p c m", p=128)
            wgr = wgw_e.rearrange("(c p) m -> p c m", p=128)
            cosr = load_tiles(cos_e, T, 1)[0]
            sinr = load_tiles(sin_e, T, 1)[0]
            cmask = load_tiles(cm_e, QT, CPQ)
            ones64 = cpool.tile([1, 64], BF16, tag="ones64", name="ones64")
            nc.vector.memset(ones64[:], 1.0)
            ident = load_tiles(id_e, 128, 1)[0]
            epsc = cpool.tile([128, 1], F32, tag="epsc", name="epsc")
            nc.vector.memset(epsc[:], EPS)

            ar1_in = dpool.tile([T, D], BF16, name="ar1_in")
            ar1_out = dpool.tile([T, D], BF16, name="ar1_out")
            ar2_in = dpool.tile([T, D], BF16, name="ar2_in")
            ar2_out = dpool.tile([T, D], BF16, name="ar2_out")

            # ---- persistent activation tiles ----
            xnf = [
                fmpool.tile([128, T], BF16, tag=f"fm{d}", name=f"xnf{d}")
                for d in range(DC)
            ]
            q_sb = [
                qkpool.tile([128, T], BF16, tag=f"qk{i}", name=f"q{i}")
                for i in range(2)
            ]
            k_sb = [
                qkpool.tile([128, T], BF16, tag=f"qk{i + 2}", name=f"k{i}")
                for i in range(2)
            ]
            O_sb = [
                qkpool.tile([128, T], BF16, tag=f"qk{i + 4}", name=f"O{i}")
                for i in range(2)
            ]
            On_sb = [
                qkpool.tile([128, T], BF16, tag=f"qk{i + 6}", name=f"On{i}")
                for i in range(2)
            ]
            v_aug = [
                vpool.tile([128, HPC, DH + 1], BF16, tag=f"va{ti}", name=f"va{ti}")
                for ti in range(TT)
            ]

            # ---- helpers ----
            def norm_into_fm(xt, fm_tiles, ti):
                """rmsnorm the f32 token tile xt, write bf16 feature-major."""
                ss = spool.tile([128, 1], F32, tag="ss", name="ss")
                sq = xnpool.tile([128, D], BF16, tag="sq", name="sq", bufs=2)
                nc.scalar.activation(
                    out=sq[:], in_=xt[:], func=AF.Square, accum_out=ss[:]
                )
                sr = spool.tile([128, 1], F32, tag="sr", name="sr")
                nc.scalar.activation(
                    out=sr[:], in_=ss[:], func=AF.Sqrt, bias=epsc[:], scale=1.0 / D
                )
                s1 = spool.tile([128, 1], F32, tag="s1", name="s1")
                nc.vector.reciprocal(s1[:], sr[:])
                xn = xnpool.tile([128, D], BF16, tag="xn", name="xn")
                nc.vector.tensor_scalar_mul(xn[:], xt[:], s1[:])
                for di in range(DC):
                    tp = psS.tile([128, 128], BF16, tag="tp", name="tp", bufs=2)
                    nc.tensor.transpose(
                        tp[:], xn[:, di * 128 : (di + 1) * 128], ident[:]
                    )
                    nc.any.tensor_copy(
                        fm_tiles[di][:, ti * 128 : (ti + 1) * 128], tp[:]
                    )

            # ---- stage A: norm1 + transpose ----
            for ti in range(TT):
                xt = xpool.tile([128, D], F32, tag="xt", name="xt")
                nc.sync.dma_start(xt[:], x_e[ti * 128 : (ti + 1) * 128, :])
                norm_into_fm(xt, xnf, ti)

            # ---- stage B: qkv + rope (chunk-ordered) ----
            def qk_t4(t4):
                tsl = slice(t4 * QT, (t4 + 1) * QT)
                for m in range(4):  # q01 q23 k01 k23
                    dst = q_sb[m] if m < 2 else k_sb[m - 2]
                    ps = psA.tile(
                        [128, 512 if T >= 512 else T], F32, tag="ps", name="ps"
                    )
                    for dc in range(DC):
                        nc.tensor.matmul(
                            ps[:, :QT],
                            qkw[dc][:, m * 128 : (m + 1) * 128],
                            xnf[dc][:, tsl],
                            start=(dc == 0),
                            stop=(dc == DC - 1),
                        )
                    qb = rpool.tile([128, QT], BF16, tag="qb", name="qb")
                    nc.scalar.copy(qb[:], ps[:, :QT])
                    rot = rpool.tile([128, QT], BF16, tag="rot", name="rot")
                    for hb in (0, 64):
                        nc.vector.tensor_scalar_mul(
                            rot[hb : hb + 32, :], qb[hb + 32 : hb + 64, :], -1.0
                        )
                        nc.vector.tensor_copy(
                            rot[hb + 32 : hb + 64, :], qb[hb : hb + 32, :]
                        )
                    t1 = rpool.tile([128, QT], BF16, tag="t1", name="t1")
                    nc.vector.tensor_mul(t1[:], qb[:], cosr[:, tsl])
                    t2 = rpool.tile([128, QT], BF16, tag="t2", name="t2")
                    nc.vector.tensor_mul(t2[:], rot[:], sinr[:, tsl])
                    nc.vector.tensor_add(dst[:, tsl], t1[:], t2[:])

            def v_chunk(t4):
                for ti in range(t4 * CPQ, (t4 + 1) * CPQ):
                    ps = psS.tile([128, VF], F32, tag="tp", name="psv")
                    for dc in range(DC):
                        nc.tensor.matmul(
                            ps[:],
                            xnf[dc][:, ti * 128 : (ti + 1) * 128],
                            vw[dc][:],
                            start=(dc == 0),
                            stop=(dc == DC - 1),
                        )
                    va = v_aug[ti]
                    nc.vector.tensor_copy(
                        va[:, :, 0:DH], ps.rearrange("p (h d) -> p h d", h=HPC)
                    )
                    nc.vector.memset(va[:, :, DH : DH + 1], 1.0)

            # ---- stage C/D: attention, o-proj, chunked AR1 (pipelined) ----
            def attn_qtile(qt):
                tsl = slice(qt * QT, (qt + 1) * QT)
                ncks = CPQ * (qt + 1)
                rinvb = spool.tile(
                    [1, HPC * QT], BF16, tag="rinvb", name="rinvb", bufs=2
                )
                for hp in range(2):
                    opsP = [
                        psO.tile([DH + 1, QT], F32, tag="pso", name=f"ops{i}")
                        for i in range(2)
                    ]
                    for ck in range(ncks):
                        pts = []
                        for i in range(2):
                            hb = i * 64
                            sp = psA.tile(
                                [128, 512 if T >= 512 else T], F32, tag="ps", name="sp"
                            )
                            nc.tensor.matmul(
                                sp[:, :QT],
                                k_sb[hp][hb : hb + DH, ck * 128 : (ck + 1) * 128],
                                q_sb[hp][hb : hb + DH, tsl],
                                start=True,
                                stop=True,
                            )
                            pt = wpool.tile(
                                [128, QT], BF16, tag="pt", name="pt", bufs=4
                            )
                            j = ck - CPQ * qt
                            if j > 0:
                                lo = j * 128
                                nc.vector.memset(pt[:, :lo], 0.0)
                                nc.scalar.activation(
                                    out=pt[:, lo:],
                                    in_=sp[:, lo:QT],
                                    func=AF.Exp,
                                    scale=0.125,
                                )
                                nc.vector.tensor_mul(
                                    pt[:, lo:], pt[:, lo:], cmask[j][:, lo:]
                                )
                            else:
                                nc.scalar.activation(
                                    out=pt[:], in_=sp[:, :QT], func=AF.Exp, scale=0.125
                                )
                                if j == 0:
                                    nc.vector.tensor_mul(pt[:], pt[:], cmask[j][:])
                            pts.append(pt)
                        for i in range(2):
                            nc.tensor.matmul(
                                opsP[i][:],
                                v_aug[ck][:, 2 * hp + i, :],
                                pts[i][:],
                                start=(ck == 0),
                                stop=(ck == ncks - 1),
                            )
                    for i in range(2):
                        h = 2 * hp + i
                        ops = opsP[i]
                        rsc = spool.tile([1, QT], F32, tag="rsc", name="rsc", bufs=2)
                        nc.vector.reciprocal(rsc[:], ops[DH : DH + 1, :])
                        nc.vector.tensor_copy(
                            rinvb[:, h * QT : (h + 1) * QT], rsc[:]
                        )
                        nc.scalar.copy(
                            O_sb[hp][i * 64 : i * 64 + DH, tsl], ops[0:DH, :]
                        )
                return rinvb

            def normalize_qt(qt, rinvb):
                tsl = slice(qt * QT, (qt + 1) * QT)
                for ot in range(2):
                    bb = psA.tile(
                        [128, 512 if T >= 512 else T], F32, tag="ps", name="bb"
                    )
                    for i in range(2):
                        h = 2 * ot + i
                        nc.tensor.matmul(
                            bb[i * 64 : (i + 1) * 64, :QT],
                            ones64[:],
                            rinvb[:, h * QT : (h + 1) * QT],
                            start=True,
                            stop=True,
                        )
                    nc.vector.tensor_mul(
                        On_sb[ot][:, tsl], O_sb[ot][:, tsl], bb[:, :QT]
                    )

            def oproj_ar1(qt):  # o-proj only; AR fired separately
                for ti in range(qt * CPQ, (qt + 1) * CPQ):
                    ob = wpool.tile([128, D], BF16, tag="ob", name="ob", bufs=3)
                    xo = xpool.tile([128, D], F32, tag="xt", name="xo")
                    nc.sync.dma_start(xo[:], x_e[ti * 128 : (ti + 1) * 128, :])
                    for nt in range(NT):
                        ps = psA.tile([128, 512], F32, tag="ps", name="ps")
                        for c in range(VF // 128):
                            nc.tensor.matmul(
                                ps[:, :512],
                                On_sb[c][:, ti * 128 : (ti + 1) * 128],
                                ow[c][:, nt * 512 : (nt + 1) * 512],
                                start=(c == 0),
                                stop=(c == VF // 128 - 1),
                            )
                        nc.vector.scalar_tensor_tensor(
                            ob[:, nt * 512 : (nt + 1) * 512],
                            xo[:, nt * 512 : (nt + 1) * 512],
                            1.0 / TP,
                            ps[:, :512],
                            ALU.mult,
                            ALU.add,
                        )
                    nc.sync.dma_start(ar1_in[ti * 128 : (ti + 1) * 128, :], ob[:])

            def ar1_fire(lo, hi):
                nc.gpsimd.collective_compute(
                    "AllReduce",
                    ALU.add,
                    ins=[ar1_in[lo:hi, :].opt()],
                    outs=[ar1_out[lo:hi, :].opt()],
                    replica_groups=groups,
                )

            hnf = [
                fmpool.tile([128, T], BF16, tag=f"fm{d}", name=f"hnf{d}")
                for d in range(DC)
            ]

            def resid_chunk(k):
                for ti in range(k * CPQ, (k + 1) * CPQ):
                    h1 = xpool.tile([128, D], F32, tag="at", name="h1")
                    nc.gpsimd.dma_start(
                        h1[:], ar1_out[ti * 128 : (ti + 1) * 128, :]
                    )
                    norm_into_fm(h1, hnf, ti)

            if NQ == 1:
                qk_t4(0)
                v_chunk(0)
                rinvb = attn_qtile(0)
                normalize_qt(0, rinvb)
                oproj_ar1(0)
                ar1_fire(0, T)
                resid_chunk(0)
            else:
                for t4 in range(NQ):
                    qk_t4(t4)
                    v_chunk(t4)
                for qt in range(NQ):
                    rinvb = attn_qtile(qt)
                    normalize_qt(qt, rinvb)
                    oproj_ar1(qt)
                    ar1_fire(qt * QT, (qt + 1) * QT)
                    if qt >= 2:
                        resid_chunk(qt - 2)

            # ---- stage E: MLP (chunk-pipelined) ----
            w2w = load_tiles(w2w_e, D, MIDC)
            a_fm = [
                qkpool.tile([128, T], BF16, tag=f"qk{d}", name=f"a{d}")
                for d in range(MIDC)
            ]

            def mlp_t4(t4):
                tsl = slice(t4 * QT, (t4 + 1) * QT)
                for mc in range(MIDC):
                    msl = slice(mc * 128, (mc + 1) * 128)
                    wg_mc = wpool.tile(
                        [128, DC, 128], BF16, tag="wgs", name="wg_mc", bufs=2
                    )
                    nc.sync.dma_start(wg_mc[:], wgr[:, :, msl])
                    w1_mc = wpool.tile(
                        [128, DC, 128], BF16, tag="w1s", name="w1_mc", bufs=2
                    )
                    nc.sync.dma_start(w1_mc[:], w1r[:, :, msl])
                    psg = psA.tile(
                        [128, 512 if T >= 512 else T], F32, tag="ps", name="psg"
                    )
                    for dc in range(DC):
                        nc.tensor.matmul(
                            psg[:, :QT],
                            wg_mc[:, dc, :],
                            hnf[dc][:, tsl],
                            start=(dc == 0),
                            stop=(dc == DC - 1),
                        )
                    g_sb = wpool.tile([128, QT], BF16, tag="g", name="g", bufs=2)
                    if use_silu:
                        nc.scalar.activation(
                            out=g_sb[:], in_=psg[:, :QT], func=AF.Silu
                        )
                    else:
                        sg = wpool.tile([128, QT], F32, tag="sg", name="sg", bufs=2)
                        nc.scalar.activation(
                            out=sg[:], in_=psg[:, :QT], func=AF.Sigmoid
                        )
                        nc.vector.tensor_mul(g_sb[:], sg[:], psg[:, :QT])
                    psu = psA.tile(
                        [128, 512 if T >= 512 else T], F32, tag="ps", name="psu"
                    )
                    for dc in range(DC):
                        nc.tensor.matmul(
                            psu[:, :QT],
                            w1_mc[:, dc, :],
                            hnf[dc][:, tsl],
                            start=(dc == 0),
                            stop=(dc == DC - 1),
                        )
                    nc.vector.tensor_mul(a_fm[mc][:, tsl], g_sb[:], psu[:, :QT])

            def w2_ar2(t4):
                for ti in range(t4 * CPQ, (t4 + 1) * CPQ):
                    ob = wpool.tile([128, D], BF16, tag="ob", name="ob", bufs=3)
                    h1t = xpool.tile([128, D], F32, tag="at", name="h1t")
                    nc.gpsimd.dma_start(
                        h1t[:], ar1_out[ti * 128 : (ti + 1) * 128, :]
                    )
                    for nt in range(NT):
                        ps = psA.tile([128, 512], F32, tag="ps", name="ps")
                        for mc in range(MIDC):
                            nc.tensor.matmul(
                                ps[:, :512],
                                a_fm[mc][:, ti * 128 : (ti + 1) * 128],
                                w2w[mc][:, nt * 512 : (nt + 1) * 512],
                                start=(mc == 0),
                                stop=(mc == MIDC - 1),
                            )
                        nc.vector.scalar_tensor_tensor(
                            ob[:, nt * 512 : (nt + 1) * 512],
                            h1t[:, nt * 512 : (nt + 1) * 512],
                            1.0 / TP,
                            ps[:, :512],
                            ALU.mult,
                            ALU.add,
                        )
                    nc.sync.dma_start(ar2_in[ti * 128 : (ti + 1) * 128, :], ob[:])

            def ar2_fire(lo, hi):
                nc.gpsimd.collective_compute(
                    "AllReduce",
                    ALU.add,
                    ins=[ar2_in[lo:hi, :].opt()],
                    outs=[ar2_out[lo:hi, :].opt()],
                    replica_groups=groups,
                )

            def final_chunk(k):
                nc.gpsimd.dma_start(
                    out_e[k * QT : (k + 1) * QT, :],
                    ar2_out[k * QT : (k + 1) * QT, :],
                )

            if NQ == 1:
                mlp_t4(0)
                w2_ar2(0)
                ar2_fire(0, T)
                final_chunk(0)
            else:
                mlp_t4(0)
                w2_ar2(0)
                ar2_fire(0, QT)
                resid_chunk(2)
                mlp_t4(1)
                w2_ar2(1)
                ar2_fire(QT, 2 * QT)
                resid_chunk(3)
                mlp_t4(2)
                w2_ar2(2)
                ar2_fire(2 * QT, 3 * QT)
                final_chunk(0)
                mlp_t4(3)
                w2_ar2(3)
                ar2_fire(3 * QT, 3 * QT + QT // 2)
                final_chunk(1)
                ar2_fire(3 * QT + QT // 2, 4 * QT)
                final_chunk(2)
                final_chunk(3)

    nc.compile()
    return nc


def make_in_maps(x, n1_w, n2_w, qkv_w, o_w, w1_w, wg_w, w2_w, T):
    QT = min(512, T)
    CPQ = QT // 128
    half = DH // 2
    freqs = np.arange(half, dtype=np.float64) / half
    theta = 1.0 / ROPE_BASE**freqs
    ang = np.arange(T, dtype=np.float64)[:, None] * theta[None, :]  # [T, 32]
    p = np.arange(128) % half
    cosr = np.cos(ang)[:, p].T.astype(BF)  # [128, T]
    sinr = np.sin(ang)[:, p].T.astype(BF)
    cm = np.zeros((CPQ * 128, QT), dtype=BF)
    for j in range(CPQ):
        tk = np.arange(128)[:, None]
        tq = np.arange(QT)[None, :]
        cm[j * 128 : (j + 1) * 128] = (tq >= j * 128 + tk).astype(BF)

    in_maps = []
    for c in range(8):
        b, r = c // 4, c % 4
        qs = slice(r * VF, (r + 1) * VF)
        qr = qkv_w[0 * D :][qs] * n1_w[None, :]
        kr = qkv_w[1 * D :][qs] * n1_w[None, :]
        vr = qkv_w[2 * D :][qs] * n1_w[None, :]
        ms = slice(r * MID, (r + 1) * MID)
        in_maps.append(
            {
                "x": np.ascontiguousarray(x[b, :T], np.float32),
                "qkw_t": np.ascontiguousarray(
                    np.concatenate([qr, kr], 0).T.astype(BF)
                ),
                "vw_m": np.ascontiguousarray(vr.T.astype(BF)),
                "ow_m": np.ascontiguousarray(o_w[:, qs].T.astype(BF)),
                "w1w_t": np.ascontiguousarray(
                    (w1_w[ms] * n2_w[None, :]).T.astype(BF)
                ),
                "wgw_t": np.ascontiguousarray(
                    (wg_w[ms] * n2_w[None, :]).T.astype(BF)
                ),
                "w2w_m": np.ascontiguousarray(w2_w[:, ms].T.astype(BF)),
                "cosr": cosr,
                "sinr": sinr,
                "cmask": cm,
                "ident": np.eye(128, dtype=BF),
            }
        )
    return in_maps


_CACHE = {}


def _get_nc(T):
    if T not in _CACHE:
        _CACHE[T] = build_nc(T)
    return _CACHE[T]


def run(inputs, T=2048, trace=False):
    nc = _get_nc(T)
    in_maps = make_in_maps(T=T, **inputs)
    res = run_bass_kernel_spmd(nc, in_maps, core_ids=list(range(8)), trace=trace)
    out = np.stack([res.results[0]["out"], res.results[4]["out"]])
    return out, res


def kernel(**inputs):
    out, _ = run(inputs, T=2048)
    return out


# revision 30
# speedup vs baseline: 1.0274x; 1.0274x over previous
"""Distributed Trainium2 kernel for a dense transformer block.

Reference computation (per batch):
  x = x + o_proj(attn(rope(qkv(rmsnorm(x))), causal)) ; x = x + w2(silu(wg(rmsnorm(x))) * w1(rmsnorm(x)))

Sharding: DP=2 on batch x TP=4 on heads / MLP rows (Megatron).
Cores 0-3 handle batch 0, cores 4-7 batch 1. Within a group, rank r owns
heads 4r..4r+3 and MLP rows 1024r..1024(r+1). Two bf16 AllReduces per
group, each chunked into NQ pieces and software-pipelined against compute.
"""

import sys

sys.path.insert(0, "/opt/trn_rl_repo")

import numpy as np
import ml_dtypes

import concourse.bass as bass
import concourse.bacc as bacc
import concourse.mybir as mybir
import concourse.tile as tile
from concourse.bass_utils import run_bass_kernel_spmd

BF = ml_dtypes.bfloat16
F32 = mybir.dt.float32
BF16 = mybir.dt.bfloat16

D = 1024
NH = 16
DH = 64
MULT = 4
EPS = 1e-5
ROPE_BASE = 10000.0
B = 2
TP = 4  # tensor-parallel ranks per group
HPC = NH // TP  # heads per core = 4
QKF = 2 * HPC * DH  # q+k shard features = 512
VF = HPC * DH  # v shard features = 256
MID = MULT * D // TP  # mlp rows per core = 1024
AF = mybir.ActivationFunctionType
ALU = mybir.AluOpType


def build_nc(T, use_silu=False):
    """Build the SPMD graph for one core (token count T per batch)."""
    DC = D // 128  # d chunks = 8
    TT = T // 128  # token tiles
    QT = min(512, T)  # q-tile width == AR chunk width
    NQ = T // QT
    CPQ = QT // 128  # 128-token tiles per chunk
    MIDC = MID // 128  # mlp row chunks = 8
    NT = D // 512

    nc = bacc.Bacc("TRN2", target_bir_lowering=False, debug=False, num_devices=8)

    x_e = nc.dram_tensor("x", [T, D], F32, kind="ExternalInput")
    qkw_e = nc.dram_tensor("qkw_t", [D, QKF], BF16, kind="ExternalInput")
    vw_e = nc.dram_tensor("vw_m", [D, VF], BF16, kind="ExternalInput")
    ow_e = nc.dram_tensor("ow_m", [VF, D], BF16, kind="ExternalInput")
    w1w_e = nc.dram_tensor("w1w_t", [D, MID], BF16, kind="ExternalInput")
    wgw_e = nc.dram_tensor("wgw_t", [D, MID], BF16, kind="ExternalInput")
    w2w_e = nc.dram_tensor("w2w_m", [MID, D], BF16, kind="ExternalInput")
    cos_e = nc.dram_tensor("cosr", [128, T], BF16, kind="ExternalInput")
    sin_e = nc.dram_tensor("sinr", [128, T], BF16, kind="ExternalInput")
    cm_e = nc.dram_tensor("cmask", [CPQ * 128, QT], BF16, kind="ExternalInput")
    id_e = nc.dram_tensor("ident", [128, 128], BF16, kind="ExternalInput")
    out_e = nc.dram_tensor("out", [T, D], F32, kind="ExternalOutput")

    groups = [[0, 1, 2, 3], [4, 5, 6, 7]]

    with tile.TileContext(nc) as tc:
        with (
            tc.tile_pool(name="const", bufs=1) as cpool,
            tc.tile_pool(name="actfm", bufs=1) as fmpool,
            tc.tile_pool(name="qko", bufs=1) as qkpool,
            tc.tile_pool(name="vaug", bufs=1) as vpool,
            tc.tile_pool(name="xin", bufs=2) as xpool,
            tc.tile_pool(name="xnb", bufs=3) as xnpool,
            tc.tile_pool(name="work", bufs=4) as wpool,
            tc.tile_pool(name="rope", bufs=2) as rpool,
            tc.tile_pool(name="stats", bufs=8) as spool,
            tc.tile_pool(name="psA", bufs=4, space="PSUM") as psA,
            tc.tile_pool(name="psO", bufs=2, space="PSUM") as psO,
            tc.tile_pool(name="psS", bufs=2, space="PSUM") as psS,
            tc.tile_pool(name="dram", bufs=1, space="DRAM") as dpool,
        ):
            # ---- resident weights / tables ----
            def load_tiles(src, width, n, dt=BF16):
                ts = []
                for i in range(n):
                    t = cpool.tile(
                        [128, width], dt, tag=f"{src.name}_{i}", name=f"{src.name}_{i}"
                    )
                    nc.sync.dma_start(t[:], src[i * 128 : (i + 1) * 128, :])
                    ts.append(t)
                return ts

            qkw = load_tiles(qkw_e, QKF, DC)
            vw = load_tiles(vw_e, VF, DC)
            ow = load_tiles(ow_e, D, VF // 128)
            w1r = w1w_e.rearrange("(c p) m -> p c m", p=128)
            wgr = wgw_e.rearrange("(c p) m -> p c m", p=128)
            cosr = load_tiles(cos_e, T, 1)[0]
            sinr = load_tiles(sin_e, T, 1)[0]
            cmask = load_tiles(cm_e, QT, CPQ)
            ones64 = cpool.tile([1, 64], BF16, tag="ones64", name="ones64")
            nc.vector.memset(ones64[:], 1.0)
            ident = load_tiles(id_e, 128, 1)[0]
            epsc = cpool.tile([128, 1], F32, tag="epsc", name="epsc")
            nc.vector.memset(epsc[:], EPS)

            ar1_in = dpool.tile([T, D], BF16, name="ar1_in")
            ar1_out = dpool.tile([T, D], BF16, name="ar1_out")
            ar2_in = dpool.tile([T, D], BF16, name="ar2_in")
            ar2_out = dpool.tile([T, D], BF16, name="ar2_out")

            # ---- persistent activation tiles ----
            xnf = [
                fmpool.tile([128, T], BF16, tag=f"fm{d}", name=f"xnf{d}")
                for d in range(DC)
            ]
            q_sb = [
                qkpool.tile([128, T], BF16, tag=f"qk{i}", name=f"q{i}")
                for i in range(2)
            ]
            k_sb = [
                qkpool.tile([128, T], BF16, tag=f"qk{i + 2}", name=f"k{i}")
                for i in range(2)
            ]
            O_sb = [
                qkpool.tile([128, T], BF16, tag=f"qk{i + 4}", name=f"O{i}")
                for i in range(2)
            ]
            On_sb = [
                qkpool.tile([128, T], BF16, tag=f"qk{i + 6}", name=f"On{i}")
                for i in range(2)
            ]
            v_aug = [
                vpool.tile([128, HPC, DH + 1], BF16, tag=f"va{ti}", name=f"va{ti}")
                for ti in range(TT)
            ]

            # ---- helpers ----
            def norm_into_fm(xt, fm_tiles, ti):
                """rmsnorm the f32 token tile xt, write bf16 feature-major."""
                ss = spool.tile([128, 1], F32, tag="ss", name="ss")
                sq = xnpool.tile([128, D], BF16, tag="sq", name="sq", bufs=2)
                nc.scalar.activation(
                    out=sq[:], in_=xt[:], func=AF.Square, accum_out=ss[:]
                )
                sr = spool.tile([128, 1], F32, tag="sr", name="sr")
                nc.scalar.activation(
                    out=sr[:], in_=ss[:], func=AF.Sqrt, bias=epsc[:], scale=1.0 / D
                )
                s1 = spool.tile([128, 1], F32, tag="s1", name="s1")
                nc.vector.reciprocal(s1[:], sr[:])
                xn = xnpool.tile([128, D], BF16, tag="xn", name="xn")
                nc.vector.tensor_scalar_mul(xn[:], xt[:], s1[:])
                for di in range(DC):
                    tp = psS.tile([128, 128], BF16, tag="tp", name="tp", bufs=2)
                    nc.tensor.transpose(
                        tp[:], xn[:, di * 128 : (di + 1) * 128], ident[:]
                    )
                    nc.any.tensor_copy(
                        fm_tiles[di][:, ti * 128 : (ti + 1) * 128], tp[:]
                    )

            # ---- stage A: norm1 + transpose ----
            for ti in range(TT):
                xt = xpool.tile([128, D], F32, tag="xt", name="xt")
                nc.sync.dma_start(xt[:], x_e[ti * 128 : (ti + 1) * 128, :])
                norm_into_fm(xt, xnf, ti)

            # ---- stage B: qkv + rope (chunk-ordered) ----
            def qk_t4(t4):
                tsl = slice(t4 * QT, (t4 + 1) * QT)
                for m in range(4):  # q01 q23 k01 k23
                    dst = q_sb[m] if m < 2 else k_sb[m - 2]
                    ps = psA.tile(
                        [128, 512 if T >= 512 else T], F32, tag="ps", name="ps"
                    )
                    for dc in range(DC):
                        nc.tensor.matmul(
                            ps[:, :QT],
                            qkw[dc][:, m * 128 : (m + 1) * 128],
                            xnf[dc][:, tsl],
                            start=(dc == 0),
                            stop=(dc == DC - 1),
                        )
                    qb = rpool.tile([128, QT], BF16, tag="qb", name="qb")
                    nc.scalar.copy(qb[:], ps[:, :QT])
                    rot = rpool.tile([128, QT], BF16, tag="rot", name="rot")
                    for hb in (0, 64):
                        nc.vector.tensor_scalar_mul(
                            rot[hb : hb + 32, :], qb[hb + 32 : hb + 64, :], -1.0
                        )
                        nc.vector.tensor_copy(
                            rot[hb + 32 : hb + 64, :], qb[hb : hb + 32, :]
                        )
                    t1 = rpool.tile([128, QT], BF16, tag="t1", name="t1")
                    nc.vector.tensor_mul(t1[:], qb[:], cosr[:, tsl])
                    t2 = rpool.tile([128, QT], BF16, tag="t2", name="t2")
                    nc.vector.tensor_mul(t2[:], rot[:], sinr[:, tsl])
                    nc.vector.tensor_add(dst[:, tsl], t1[:], t2[:])

            def v_chunk(t4):
                for ti in range(t4 * CPQ, (t4 + 1) * CPQ):
                    ps = psS.tile([128, VF], F32, tag="tp", name="psv")
                    for dc in range(DC):
                        nc.tensor.matmul(
                            ps[:],
                            xnf[dc][:, ti * 128 : (ti + 1) * 128],
                            vw[dc][:],
                            start=(dc == 0),
                            stop=(dc == DC - 1),
                        )
                    va = v_aug[ti]
                    nc.vector.tensor_copy(
                        va[:, :, 0:DH], ps.rearrange("p (h d) -> p h d", h=HPC)
                    )
                    nc.vector.memset(va[:, :, DH : DH + 1], 1.0)

            # ---- stage C/D: attention, o-proj, chunked AR1 (pipelined) ----
            def attn_qtile(qt):
                tsl = slice(qt * QT, (qt + 1) * QT)
                ncks = CPQ * (qt + 1)
                rinvb = spool.tile(
                    [1, HPC * QT], BF16, tag="rinvb", name="rinvb", bufs=2
                )
                for hp in range(2):
                    opsP = [
                        psO.tile([DH + 1, QT], F32, tag="pso", name=f"ops{i}")
                        for i in range(2)
                    ]
                    for ck in range(ncks):
                        pts = []
                        for i in range(2):
                            hb = i * 64
                            sp = psA.tile(
                                [128, 512 if T >= 512 else T], F32, tag="ps", name="sp"
                            )
                            nc.tensor.matmul(
                                sp[:, :QT],
                                k_sb[hp][hb : hb + DH, ck * 128 : (ck + 1) * 128],
                                q_sb[hp][hb : hb + DH, tsl],
                                start=True,
                                stop=True,
                            )
                            pt = wpool.tile(
                                [128, QT], BF16, tag="pt", name="pt", bufs=4
                            )
                            j = ck - CPQ * qt
                            if j > 0:
                                lo = j * 128
                                nc.vector.memset(pt[:, :lo], 0.0)
                                nc.scalar.activation(
                                    out=pt[:, lo:],
                                    in_=sp[:, lo:QT],
                                    func=AF.Exp,
                                    scale=0.125,
                                )
                                nc.vector.tensor_mul(
                                    pt[:, lo:], pt[:, lo:], cmask[j][:, lo:]
                                )
                            else:
                                nc.scalar.activation(
                                    out=pt[:], in_=sp[:, :QT], func=AF.Exp, scale=0.125
                                )
                                if j == 0:
                                    nc.vector.tensor_mul(pt[:], pt[:], cmask[j][:])
                            pts.append(pt)
                        for i in range(2):
                            nc.tensor.matmul(
                                opsP[i][:],
                                v_aug[ck][:, 2 * hp + i, :],
                                pts[i][:],
                                start=(ck == 0),
                                stop=(ck == ncks - 1),
                            )
                    for i in range(2):
                        h = 2 * hp + i
                        ops = opsP[i]
                        rsc = spool.tile([1, QT], F32, tag="rsc", name="rsc", bufs=2)
                        nc.vector.reciprocal(rsc[:], ops[DH : DH + 1, :])
                        nc.vector.tensor_copy(
                            rinvb[:, h * QT : (h + 1) * QT], rsc[:]
                        )
                        nc.scalar.copy(
                            O_sb[hp][i * 64 : i * 64 + DH, tsl], ops[0:DH, :]
                        )
                return rinvb

            def normalize_qt(qt, rinvb):
                tsl = slice(qt * QT, (qt + 1) * QT)
                for ot in range(2):
                    bb = psA.tile(
                        [128, 512 if T >= 512 else T], F32, tag="ps", name="bb"
                    )
                    for i in range(2):
                        h = 2 * ot + i
                        nc.tensor.matmul(
                            bb[i * 64 : (i + 1) * 64, :QT],
                            ones64[:],
                            rinvb[:, h * QT : (h + 1) * QT],
                            start=True,
                            stop=True,
                        )
                    nc.vector.tensor_mul(
                        On_sb[ot][:, tsl], O_sb[ot][:, tsl], bb[:, :QT]
                    )

            def oproj_ar1(qt):  # o-proj only; AR fired separately
                for ti in range(qt * CPQ, (qt + 1) * CPQ):
                    ob = wpool.tile([128, D], BF16, tag="ob", name="ob", bufs=3)
                    xo = xpool.tile([128, D], F32, tag="xt", name="xo")
                    nc.sync.dma_start(xo[:], x_e[ti * 128 : (ti + 1) * 128, :])
                    for nt in range(NT):
                        ps = psA.tile([128, 512], F32, tag="ps", name="ps")
                        for c in range(VF // 128):
                            nc.tensor.matmul(
                                ps[:, :512],
                                On_sb[c][:, ti * 128 : (ti + 1) * 128],
                                ow[c][:, nt * 512 : (nt + 1) * 512],
                                start=(c == 0),
                                stop=(c == VF // 128 - 1),
                            )
                        nc.vector.scalar_tensor_tensor(
                            ob[:, nt * 512 : (nt + 1) * 512],
                            xo[:, nt * 512 : (nt + 1) * 512],
                            1.0 / TP,
                            ps[:, :512],
                            ALU.mult,
                            ALU.add,
                        )
                    nc.sync.dma_start(ar1_in[ti * 128 : (ti + 1) * 128, :], ob[:])

            def ar1_fire(lo, hi):
                nc.gpsimd.collective_compute(
                    "AllReduce",
                    ALU.add,
                    ins=[ar1_in[lo:hi, :].opt()],
                    outs=[ar1_out[lo:hi, :].opt()],
                    replica_groups=groups,
                )

            hnf = [
                fmpool.tile([128, T], BF16, tag=f"fm{d}", name=f"hnf{d}")
                for d in range(DC)
            ]

            def resid_chunk(k):
                for ti in range(k * CPQ, (k + 1) * CPQ):
                    h1 = xpool.tile([128, D], F32, tag="at", name="h1")
                    nc.gpsimd.dma_start(
                        h1[:], ar1_out[ti * 128 : (ti + 1) * 128, :]
                    )
                    norm_into_fm(h1, hnf, ti)

            if NQ == 1:
                qk_t4(0)
                v_chunk(0)
                rinvb = attn_qtile(0)
                normalize_qt(0, rinvb)
                oproj_ar1(0)
                ar1_fire(0, T)
                resid_chunk(0)
            else:
                for t4 in range(NQ):
                    qk_t4(t4)
                    v_chunk(t4)
                for qt in range(NQ):
                    rinvb = attn_qtile(qt)
                    normalize_qt(qt, rinvb)
                    oproj_ar1(qt)
                    ar1_fire(qt * QT, (qt + 1) * QT)
                    if qt >= 2:
                        resid_chunk(qt - 2)

            # ---- stage E: MLP (chunk-pipelined) ----
            w2w = load_tiles(w2w_e, D, MIDC)
            a_fm = [
                qkpool.tile([128, T], BF16, tag=f"qk{d}", name=f"a{d}")
                for d in range(MIDC)
            ]

            def mlp_t4(t4):
                tsl = slice(t4 * QT, (t4 + 1) * QT)
                for mc in range(MIDC):
                    msl = slice(mc * 128, (mc + 1) * 128)
                    wg_mc = wpool.tile(
                        [128, DC, 128], BF16, tag="wgs", name="wg_mc", bufs=2
                    )
                    nc.sync.dma_start(wg_mc[:], wgr[:, :, msl])
                    w1_mc = wpool.tile(
                        [128, DC, 128], BF16, tag="w1s", name="w1_mc", bufs=2
                    )
                    nc.sync.dma_start(w1_mc[:], w1r[:, :, msl])
                    psg = psA.tile(
                        [128, 512 if T >= 512 else T], F32, tag="ps", name="psg"
                    )
                    for dc in range(DC):
                        nc.tensor.matmul(
                            psg[:, :QT],
                            wg_mc[:, dc, :],
                            hnf[dc][:, tsl],
                            start=(dc == 0),
                            stop=(dc == DC - 1),
                        )
                    g_sb = wpool.tile([128, QT], BF16, tag="g", name="g", bufs=2)
                    if use_silu:
                        nc.scalar.activation(
                            out=g_sb[:], in_=psg[:, :QT], func=AF.Silu
                        )
                    else:
                        sg = wpool.tile([128, QT], F32, tag="sg", name="sg", bufs=2)
                        nc.scalar.activation(
                            out=sg[:], in_=psg[:, :QT], func=AF.Sigmoid
                        )
                        nc.vector.tensor_mul(g_sb[:], sg[:], psg[:, :QT])
                    psu = psA.tile(
                        [128, 512 if T >= 512 else T], F32, tag="ps", name="psu"
                    )
                    for dc in range(DC):
                        nc.tensor.matmul(
                            psu[:, :QT],
                            w1_mc[:, dc, :],
                            hnf[dc][:, tsl],
                            start=(dc == 0),
                            stop=(dc == DC - 1),
                        )
                    nc.vector.tensor_mul(a_fm[mc][:, tsl], g_sb[:], psu[:, :QT])

            def w2_ar2(t4):
                for ti in range(t4 * CPQ, (t4 + 1) * CPQ):
                    ob = wpool.tile([128, D], BF16, tag="ob", name="ob", bufs=3)
                    h1t = xpool.tile([128, D], F32, tag="at", name="h1t")
                    nc.gpsimd.dma_start(
                        h1t[:], ar1_out[ti * 128 : (ti + 1) * 128, :]
                    )
                    for nt in range(NT):
                        ps = psA.tile([128, 512], F32, tag="ps", name="ps")
                        for mc in range(MIDC):
                            nc.tensor.matmul(
                                ps[:, :512],
                                a_fm[mc][:, ti * 128 : (ti + 1) * 128],
                                w2w[mc][:, nt * 512 : (nt + 1) * 512],
                                start=(mc == 0),
                                stop=(mc == MIDC - 1),
                            )
                        nc.vector.scalar_tensor_tensor(
                            ob[:, nt * 512 : (nt + 1) * 512],
                            h1t[:, nt * 512 : (nt + 1) * 512],
                            1.0 / TP,
                            ps[:, :512],
                            ALU.mult,
                            ALU.add,
                        )
                    nc.sync.dma_start(ar2_in[ti * 128 : (ti + 1) * 128, :], ob[:])

            def ar2_fire(lo, hi):
                nc.gpsimd.collective_compute(
                    "AllReduce",
                    ALU.add,
                    ins=[ar2_in[lo:hi, :].opt()],
                    outs=[ar2_out[lo:hi, :].opt()],
                    replica_groups=groups,
                )

            def final_chunk(k):
                nc.gpsimd.dma_start(
                    out_e[k * QT : (k + 1) * QT, :],
                    ar2_out[k * QT : (k + 1) * QT, :],
                )

            if NQ == 1:
                mlp_t4(0)
                w2_ar2(0)
                ar2_fire(0, T)
                final_chunk(0)
            else:
                mlp_t4(0)
                w2_ar2(0)
                ar2_fire(0, QT)
                resid_chunk(2)
                mlp_t4(1)
                w2_ar2(1)
                ar2_fire(QT, 2 * QT)
                resid_chunk(3)
                mlp_t4(2)
                w2_ar2(2)
                ar2_fire(2 * QT, 3 * QT)
                final_chunk(0)
                mlp_t4(3)
                w2_ar2(3)
                ar2_fire(3 * QT, 4 * QT)
                final_chunk(1)
                final_chunk(2)
                final_chunk(3)

    nc.compile()
    return nc


def make_in_maps(x, n1_w, n2_w, qkv_w, o_w, w1_w, wg_w, w2_w, T):
    QT = min(512, T)
    CPQ = QT // 128
    half = DH // 2
    freqs = np.arange(half, dtype=np.float64) / half
    theta = 1.0 / ROPE_BASE**freqs
    ang = np.arange(T, dtype=np.float64)[:, None] * theta[None, :]  # [T, 32]
    p = np.arange(128) % half
    cosr = np.cos(ang)[:, p].T.astype(BF)  # [128, T]
    sinr = np.sin(ang)[:, p].T.astype(BF)
    cm = np.zeros((CPQ * 128, QT), dtype=BF)
    for j in range(CPQ):
        tk = np.arange(128)[:, None]
        tq = np.arange(QT)[None, :]
        cm[j * 128 : (j + 1) * 128] = (tq >= j * 128 + tk).astype(BF)

    in_maps = []
    for c in range(8):
        b, r = c // 4, c % 4
        qs = slice(r * VF, (r + 1) * VF)
        qr = qkv_w[0 * D :][qs] * n1_w[None, :]
        kr = qkv_w[1 * D :][qs] * n1_w[None, :]
        vr = qkv_w[2 * D :][qs] * n1_w[None, :]
        ms = slice(r * MID, (r + 1) * MID)
        in_maps.append(
            {
                "x": np.ascontiguousarray(x[b, :T], np.float32),
                "qkw_t": np.ascontiguousarray(
                    np.concatenate([qr, kr], 0).T.astype(BF)
                ),
                "vw_m": np.ascontiguousarray(vr.T.astype(BF)),
                "ow_m": np.ascontiguousarray(o_w[:, qs].T.astype(BF)),
                "w1w_t": np.ascontiguousarray(
                    (w1_w[ms] * n2_w[None, :]).T.astype(BF)
                ),
                "wgw_t": np.ascontiguousarray(
                    (wg_w[ms] * n2_w[None, :]).T.astype(BF)
                ),
                "w2w_m": np.ascontiguousarray(w2_w[:, ms].T.astype(BF)),
                "cosr": cosr,
                "sinr": sinr,
                "cmask": cm,
                "ident": np.eye(128, dtype=BF),
            }
        )
    return in_maps


_CACHE = {}


def _get_nc(T):
    if T not in _CACHE:
        _CACHE[T] = build_nc(T)
    return _CACHE[T]


def run(inputs, T=2048, trace=False):
    nc = _get_nc(T)
    in_maps = make_in_maps(T=T, **inputs)
    res = run_bass_kernel_spmd(nc, in_maps, core_ids=list(range(8)), trace=trace)
    out = np.stack([res.results[0]["out"], res.results[4]["out"]])
    return out, res


def kernel(**inputs):
    out, _ = run(inputs, T=2048)
    return out


# revision 31
# speedup vs baseline: 1.0311x; 1.0036x over previous
"""Distributed Trainium2 kernel for a dense transformer block.

Reference computation (per batch):
  x = x + o_proj(attn(rope(qkv(rmsnorm(x))), causal)) ; x = x + w2(silu(wg(rmsnorm(x))) * w1(rmsnorm(x)))

Sharding: DP=2 on batch x TP=4 on heads / MLP rows (Megatron).
Cores 0-3 handle batch 0, cores 4-7 batch 1. Within a group, rank r owns
heads 4r..4r+3 and MLP rows 1024r..1024(r+1). Two bf16 AllReduces per
group, each chunked into NQ pieces and software-pipelined against compute.
"""

import sys

sys.path.insert(0, "/opt/trn_rl_repo")

import numpy as np
import ml_dtypes

import concourse.bass as bass
import concourse.bacc as bacc
import concourse.mybir as mybir
import concourse.tile as tile
from concourse.bass_utils import run_bass_kernel_spmd

BF = ml_dtypes.bfloat16
F32 = mybir.dt.float32
BF16 = mybir.dt.bfloat16

D = 1024
NH = 16
DH = 64
MULT = 4
EPS = 1e-5
ROPE_BASE = 10000.0
B = 2
TP = 4  # tensor-parallel ranks per group
HPC = NH // TP  # heads per core = 4
QKF = 2 * HPC * DH  # q+k shard features = 512
VF = HPC * DH  # v shard features = 256
MID = MULT * D // TP  # mlp rows per core = 1024
AF = mybir.ActivationFunctionType
ALU = mybir.AluOpType


def build_nc(T, use_silu=False):
    """Build the SPMD graph for one core (token count T per batch)."""
    DC = D // 128  # d chunks = 8
    TT = T // 128  # token tiles
    QT = min(512, T)  # q-tile width == AR chunk width
    NQ = T // QT
    CPQ = QT // 128  # 128-token tiles per chunk
    MIDC = MID // 128  # mlp row chunks = 8
    NT = D // 512

    nc = bacc.Bacc("TRN2", target_bir_lowering=False, debug=False, num_devices=8)

    x_e = nc.dram_tensor("x", [T, D], F32, kind="ExternalInput")
    qkw_e = nc.dram_tensor("qkw_t", [D, QKF], BF16, kind="ExternalInput")
    vw_e = nc.dram_tensor("vw_m", [D, VF], BF16, kind="ExternalInput")
    ow_e = nc.dram_tensor("ow_m", [VF, D], BF16, kind="ExternalInput")
    w1w_e = nc.dram_tensor("w1w_t", [D, MID], BF16, kind="ExternalInput")
    wgw_e = nc.dram_tensor("wgw_t", [D, MID], BF16, kind="ExternalInput")
    w2w_e = nc.dram_tensor("w2w_m", [MID, D], BF16, kind="ExternalInput")
    cos_e = nc.dram_tensor("cosr", [128, T], BF16, kind="ExternalInput")
    sin_e = nc.dram_tensor("sinr", [128, T], BF16, kind="ExternalInput")
    cm_e = nc.dram_tensor("cmask", [CPQ * 128, QT], BF16, kind="ExternalInput")
    id_e = nc.dram_tensor("ident", [128, 128], BF16, kind="ExternalInput")
    out_e = nc.dram_tensor("out", [T, D], F32, kind="ExternalOutput")

    groups = [[0, 1, 2, 3], [4, 5, 6, 7]]

    with tile.TileContext(nc) as tc:
        with (
            tc.tile_pool(name="const", bufs=1) as cpool,
            tc.tile_pool(name="actfm", bufs=1) as fmpool,
            tc.tile_pool(name="qko", bufs=1) as qkpool,
            tc.tile_pool(name="vaug", bufs=1) as vpool,
            tc.tile_pool(name="xin", bufs=2) as xpool,
            tc.tile_pool(name="xnb", bufs=3) as xnpool,
            tc.tile_pool(name="work", bufs=4) as wpool,
            tc.tile_pool(name="rope", bufs=2) as rpool,
            tc.tile_pool(name="stats", bufs=8) as spool,
            tc.tile_pool(name="psA", bufs=4, space="PSUM") as psA,
            tc.tile_pool(name="psO", bufs=2, space="PSUM") as psO,
            tc.tile_pool(name="psS", bufs=2, space="PSUM") as psS,
            tc.tile_pool(name="dram", bufs=1, space="DRAM") as dpool,
        ):
            # ---- resident weights / tables ----
            def load_tiles(src, width, n, dt=BF16):
                ts = []
                for i in range(n):
                    t = cpool.tile(
                        [128, width], dt, tag=f"{src.name}_{i}", name=f"{src.name}_{i}"
                    )
                    nc.sync.dma_start(t[:], src[i * 128 : (i + 1) * 128, :])
                    ts.append(t)
                return ts

            qkw = load_tiles(qkw_e, QKF, DC)
            vw = load_tiles(vw_e, VF, DC)
            ow = load_tiles(ow_e, D, VF // 128)
            w1r = w1w_e.rearrange("(c p) m -> p c m", p=128)
            wgr = wgw_e.rearrange("(c p) m -> p c m", p=128)
            cosr = load_tiles(cos_e, T, 1)[0]
            sinr = load_tiles(sin_e, T, 1)[0]
            cmask = load_tiles(cm_e, QT, CPQ)
            ones64 = cpool.tile([1, 64], BF16, tag="ones64", name="ones64")
            nc.vector.memset(ones64[:], 1.0)
            ident = load_tiles(id_e, 128, 1)[0]
            epsc = cpool.tile([128, 1], F32, tag="epsc", name="epsc")
            nc.vector.memset(epsc[:], EPS)

            ar1_in = dpool.tile([T, D], BF16, name="ar1_in")
            ar1_out = dpool.tile([T, D], BF16, name="ar1_out")
            ar2_in = dpool.tile([T, D], BF16, name="ar2_in")
            ar2_out = dpool.tile([T, D], BF16, name="ar2_out")

            # ---- persistent activation tiles ----
            xnf = [
                fmpool.tile([128, T], BF16, tag=f"fm{d}", name=f"xnf{d}")
                for d in range(DC)
            ]
            q_sb = [
                qkpool.tile([128, T], BF16, tag=f"qk{i}", name=f"q{i}")
                for i in range(2)
            ]
            k_sb = [
                qkpool.tile([128, T], BF16, tag=f"qk{i + 2}", name=f"k{i}")
                for i in range(2)
            ]
            O_sb = [
                qkpool.tile([128, T], BF16, tag=f"qk{i + 4}", name=f"O{i}")
                for i in range(2)
            ]
            On_sb = [
                qkpool.tile([128, T], BF16, tag=f"qk{i + 6}", name=f"On{i}")
                for i in range(2)
            ]
            v_aug = [
                vpool.tile([128, HPC, DH + 1], BF16, tag=f"va{ti}", name=f"va{ti}")
                for ti in range(TT)
            ]

            # ---- helpers ----
            def norm_into_fm(xt, fm_tiles, ti):
                """rmsnorm the f32 token tile xt, write bf16 feature-major."""
                ss = spool.tile([128, 1], F32, tag="ss", name="ss")
                sq = xnpool.tile([128, D], BF16, tag="sq", name="sq", bufs=2)
                nc.scalar.activation(
                    out=sq[:], in_=xt[:], func=AF.Square, accum_out=ss[:]
                )
                sr = spool.tile([128, 1], F32, tag="sr", name="sr")
                nc.scalar.activation(
                    out=sr[:], in_=ss[:], func=AF.Sqrt, bias=epsc[:], scale=1.0 / D
                )
                s1 = spool.tile([128, 1], F32, tag="s1", name="s1")
                nc.vector.reciprocal(s1[:], sr[:])
                xn = xnpool.tile([128, D], BF16, tag="xn", name="xn")
                nc.vector.tensor_scalar_mul(xn[:], xt[:], s1[:])
                for di in range(DC):
                    tp = psS.tile([128, 128], BF16, tag="tp", name="tp", bufs=2)
                    nc.tensor.transpose(
                        tp[:], xn[:, di * 128 : (di + 1) * 128], ident[:]
                    )
                    nc.any.tensor_copy(
                        fm_tiles[di][:, ti * 128 : (ti + 1) * 128], tp[:]
                    )

            # ---- stage A: norm1 + transpose ----
            for ti in range(TT):
                xt = xpool.tile([128, D], F32, tag="xt", name="xt")
                nc.sync.dma_start(xt[:], x_e[ti * 128 : (ti + 1) * 128, :])
                norm_into_fm(xt, xnf, ti)

            # ---- stage B: qkv + rope (chunk-ordered) ----
            def qk_t4(t4):
                tsl = slice(t4 * QT, (t4 + 1) * QT)
                for m in range(4):  # q01 q23 k01 k23
                    dst = q_sb[m] if m < 2 else k_sb[m - 2]
                    ps = psA.tile(
                        [128, 512 if T >= 512 else T], F32, tag="ps", name="ps"
                    )
                    for dc in range(DC):
                        nc.tensor.matmul(
                            ps[:, :QT],
                            qkw[dc][:, m * 128 : (m + 1) * 128],
                            xnf[dc][:, tsl],
                            start=(dc == 0),
                            stop=(dc == DC - 1),
                        )
                    qb = rpool.tile([128, QT], BF16, tag="qb", name="qb")
                    nc.scalar.copy(qb[:], ps[:, :QT])
                    rot = rpool.tile([128, QT], BF16, tag="rot", name="rot")
                    for hb in (0, 64):
                        nc.vector.tensor_scalar_mul(
                            rot[hb : hb + 32, :], qb[hb + 32 : hb + 64, :], -1.0
                        )
                        nc.vector.tensor_copy(
                            rot[hb + 32 : hb + 64, :], qb[hb : hb + 32, :]
                        )
                    t1 = rpool.tile([128, QT], BF16, tag="t1", name="t1")
                    nc.vector.tensor_mul(t1[:], qb[:], cosr[:, tsl])
                    t2 = rpool.tile([128, QT], BF16, tag="t2", name="t2")
                    nc.vector.tensor_mul(t2[:], rot[:], sinr[:, tsl])
                    nc.vector.tensor_add(dst[:, tsl], t1[:], t2[:])

            def v_chunk(t4):
                for ti in range(t4 * CPQ, (t4 + 1) * CPQ):
                    ps = psS.tile([128, VF], F32, tag="tp", name="psv")
                    for dc in range(DC):
                        nc.tensor.matmul(
                            ps[:],
                            xnf[dc][:, ti * 128 : (ti + 1) * 128],
                            vw[dc][:],
                            start=(dc == 0),
                            stop=(dc == DC - 1),
                        )
                    va = v_aug[ti]
                    nc.vector.tensor_copy(
                        va[:, :, 0:DH], ps.rearrange("p (h d) -> p h d", h=HPC)
                    )
                    nc.vector.memset(va[:, :, DH : DH + 1], 1.0)

            # ---- stage C/D: attention, o-proj, chunked AR1 (pipelined) ----
            def attn_qtile(qt):
                tsl = slice(qt * QT, (qt + 1) * QT)
                ncks = CPQ * (qt + 1)
                rinvb = spool.tile(
                    [1, HPC * QT], BF16, tag="rinvb", name="rinvb", bufs=2
                )
                for hp in range(2):
                    opsP = [
                        psO.tile([DH + 1, QT], F32, tag="pso", name=f"ops{i}")
                        for i in range(2)
                    ]
                    for ck in range(ncks):
                        pts = []
                        for i in range(2):
                            hb = i * 64
                            sp = psA.tile(
                                [128, 512 if T >= 512 else T], F32, tag="ps", name="sp"
                            )
                            nc.tensor.matmul(
                                sp[:, :QT],
                                k_sb[hp][hb : hb + DH, ck * 128 : (ck + 1) * 128],
                                q_sb[hp][hb : hb + DH, tsl],
                                start=True,
                                stop=True,
                            )
                            pt = wpool.tile(
                                [128, QT], BF16, tag="pt", name="pt", bufs=4
                            )
                            j = ck - CPQ * qt
                            if j > 0:
                                lo = j * 128
                                nc.vector.memset(pt[:, :lo], 0.0)
                                nc.scalar.activation(
                                    out=pt[:, lo:],
                                    in_=sp[:, lo:QT],
                                    func=AF.Exp,
                                    scale=0.125,
                                )
                                nc.vector.tensor_mul(
                                    pt[:, lo:], pt[:, lo:], cmask[j][:, lo:]
                                )
                            else:
                                nc.scalar.activation(
                                    out=pt[:], in_=sp[:, :QT], func=AF.Exp, scale=0.125
                                )
                                if j == 0:
                                    nc.vector.tensor_mul(pt[:], pt[:], cmask[j][:])
                            pts.append(pt)
                        for i in range(2):
                            nc.tensor.matmul(
                                opsP[i][:],
                                v_aug[ck][:, 2 * hp + i, :],
                                pts[i][:],
                                start=(ck == 0),
                                stop=(ck == ncks - 1),
                            )
                    for i in range(2):
                        h = 2 * hp + i
                        ops = opsP[i]
                        rsc = spool.tile([1, QT], F32, tag="rsc", name="rsc", bufs=2)
                        nc.vector.reciprocal(rsc[:], ops[DH : DH + 1, :])
                        nc.vector.tensor_copy(
                            rinvb[:, h * QT : (h + 1) * QT], rsc[:]
                        )
                        nc.scalar.copy(
                            O_sb[hp][i * 64 : i * 64 + DH, tsl], ops[0:DH, :]
                        )
                return rinvb

            def normalize_qt(qt, rinvb):
                tsl = slice(qt * QT, (qt + 1) * QT)
                for ot in range(2):
                    bb = psA.tile(
                        [128, 512 if T >= 512 else T], F32, tag="ps", name="bb"
                    )
                    for i in range(2):
                        h = 2 * ot + i
                        nc.tensor.matmul(
                            bb[i * 64 : (i + 1) * 64, :QT],
                            ones64[:],
                            rinvb[:, h * QT : (h + 1) * QT],
                            start=True,
                            stop=True,
                        )
                    nc.vector.tensor_mul(
                        On_sb[ot][:, tsl], O_sb[ot][:, tsl], bb[:, :QT]
                    )

            def oproj_ar1(qt):  # o-proj only; AR fired separately
                for ti in range(qt * CPQ, (qt + 1) * CPQ):
                    ob = wpool.tile([128, D], BF16, tag="ob", name="ob", bufs=3)
                    xo = xpool.tile([128, D], F32, tag="xt", name="xo")
                    nc.sync.dma_start(xo[:], x_e[ti * 128 : (ti + 1) * 128, :])
                    for nt in range(NT):
                        ps = psA.tile([128, 512], F32, tag="ps", name="ps")
                        for c in range(VF // 128):
                            nc.tensor.matmul(
                                ps[:, :512],
                                On_sb[c][:, ti * 128 : (ti + 1) * 128],
                                ow[c][:, nt * 512 : (nt + 1) * 512],
                                start=(c == 0),
                                stop=(c == VF // 128 - 1),
                            )
                        nc.vector.scalar_tensor_tensor(
                            ob[:, nt * 512 : (nt + 1) * 512],
                            xo[:, nt * 512 : (nt + 1) * 512],
                            1.0 / TP,
                            ps[:, :512],
                            ALU.mult,
                            ALU.add,
                        )
                    nc.sync.dma_start(ar1_in[ti * 128 : (ti + 1) * 128, :], ob[:])

            def ar1_fire(lo, hi):
                nc.gpsimd.collective_compute(
                    "AllReduce",
                    ALU.add,
                    ins=[ar1_in[lo:hi, :].opt()],
                    outs=[ar1_out[lo:hi, :].opt()],
                    replica_groups=groups,
                )

            hnf = [
                fmpool.tile([128, T], BF16, tag=f"fm{d}", name=f"hnf{d}")
                for d in range(DC)
            ]

            def resid_chunk(k):
                for ti in range(k * CPQ, (k + 1) * CPQ):
                    h1 = xpool.tile([128, D], F32, tag="at", name="h1")
                    nc.gpsimd.dma_start(
                        h1[:], ar1_out[ti * 128 : (ti + 1) * 128, :]
                    )
                    norm_into_fm(h1, hnf, ti)

            if NQ == 1:
                qk_t4(0)
                v_chunk(0)
                rinvb = attn_qtile(0)
                normalize_qt(0, rinvb)
                oproj_ar1(0)
                ar1_fire(0, T)
                resid_chunk(0)
            else:
                for t4 in range(NQ):
                    qk_t4(t4)
                    v_chunk(t4)
                for qt in range(NQ):
                    rinvb = attn_qtile(qt)
                    normalize_qt(qt, rinvb)
                    oproj_ar1(qt)
                    ar1_fire(qt * QT, (qt + 1) * QT)
                    if qt >= 2:
                        resid_chunk(qt - 2)

            # ---- stage E: MLP (chunk-pipelined) ----
            w2w = load_tiles(w2w_e, D, MIDC)
            a_fm = [
                qkpool.tile([128, T], BF16, tag=f"qk{d}", name=f"a{d}")
                for d in range(MIDC)
            ]

            def mlp_t4(t4):
                tsl = slice(t4 * QT, (t4 + 1) * QT)
                for mc in range(MIDC):
                    msl = slice(mc * 128, (mc + 1) * 128)
                    wg_mc = wpool.tile(
                        [128, DC, 128], BF16, tag="wgs", name="wg_mc", bufs=2
                    )
                    nc.sync.dma_start(wg_mc[:], wgr[:, :, msl])
                    w1_mc = wpool.tile(
                        [128, DC, 128], BF16, tag="w1s", name="w1_mc", bufs=2
                    )
                    nc.sync.dma_start(w1_mc[:], w1r[:, :, msl])
                    psg = psA.tile(
                        [128, 512 if T >= 512 else T], F32, tag="ps", name="psg"
                    )
                    for dc in range(DC):
                        nc.tensor.matmul(
                            psg[:, :QT],
                            wg_mc[:, dc, :],
                            hnf[dc][:, tsl],
                            start=(dc == 0),
                            stop=(dc == DC - 1),
                        )
                    g_sb = wpool.tile([128, QT], BF16, tag="g", name="g", bufs=2)
                    if use_silu:
                        nc.scalar.activation(
                            out=g_sb[:], in_=psg[:, :QT], func=AF.Silu
                        )
                    else:
                        sg = wpool.tile([128, QT], F32, tag="sg", name="sg", bufs=2)
                        nc.scalar.activation(
                            out=sg[:], in_=psg[:, :QT], func=AF.Sigmoid
                        )
                        nc.vector.tensor_mul(g_sb[:], sg[:], psg[:, :QT])
                    psu = psA.tile(
                        [128, 512 if T >= 512 else T], F32, tag="ps", name="psu"
                    )
                    for dc in range(DC):
                        nc.tensor.matmul(
                            psu[:, :QT],
                            w1_mc[:, dc, :],
                            hnf[dc][:, tsl],
                            start=(dc == 0),
                            stop=(dc == DC - 1),
                        )
                    nc.vector.tensor_mul(a_fm[mc][:, tsl], g_sb[:], psu[:, :QT])

            def w2_ar2(t4):
                for ti in range(t4 * CPQ, (t4 + 1) * CPQ):
                    ob = wpool.tile([128, D], BF16, tag="ob", name="ob", bufs=3)
                    h1t = xpool.tile([128, D], F32, tag="at", name="h1t")
                    nc.gpsimd.dma_start(
                        h1t[:], ar1_out[ti * 128 : (ti + 1) * 128, :]
                    )
                    for nt in range(NT):
                        ps = psA.tile([128, 512], F32, tag="ps", name="ps")
                        for mc in range(MIDC):
                            nc.tensor.matmul(
                                ps[:, :512],
                                a_fm[mc][:, ti * 128 : (ti + 1) * 128],
                                w2w[mc][:, nt * 512 : (nt + 1) * 512],
                                start=(mc == 0),
                                stop=(mc == MIDC - 1),
                            )
                        nc.vector.scalar_tensor_tensor(
                            ob[:, nt * 512 : (nt + 1) * 512],
                            h1t[:, nt * 512 : (nt + 1) * 512],
                            1.0 / TP,
                            ps[:, :512],
                            ALU.mult,
                            ALU.add,
                        )
                    nc.sync.dma_start(ar2_in[ti * 128 : (ti + 1) * 128, :], ob[:])

            def ar2_fire(lo, hi):
                nc.gpsimd.collective_compute(
                    "AllReduce",
                    ALU.add,
                    ins=[ar2_in[lo:hi, :].opt()],
                    outs=[ar2_out[lo:hi, :].opt()],
                    replica_groups=groups,
                )

            def final_chunk(k):
                nc.gpsimd.dma_start(
                    out_e[k * QT : (k + 1) * QT, :],
                    ar2_out[k * QT : (k + 1) * QT, :],
                )

            if NQ == 1:
                mlp_t4(0)
                w2_ar2(0)
                ar2_fire(0, T)
                final_chunk(0)
            else:
                mlp_t4(0)
                w2_ar2(0)
                ar2_fire(0, QT)
                resid_chunk(2)
                mlp_t4(1)
                w2_ar2(1)
                ar2_fire(QT, 2 * QT)
                resid_chunk(3)
                mlp_t4(2)
                w2_ar2(2)
                ar2_fire(2 * QT, 3 * QT)
                final_chunk(0)
                mlp_t4(3)
                w2_ar2(3)
                ar2_fire(3 * QT, 4 * QT)
                final_chunk(1)
                final_chunk(2)
                final_chunk(3)

    nc.compile()
    return nc


def make_in_maps(x, n1_w, n2_w, qkv_w, o_w, w1_w, wg_w, w2_w, T):
    QT = min(512, T)
    CPQ = QT // 128
    half = DH // 2
    freqs = np.arange(half, dtype=np.float64) / half
    theta = 1.0 / ROPE_BASE**freqs
    ang = np.arange(T, dtype=np.float64)[:, None] * theta[None, :]  # [T, 32]
    p = np.arange(128) % half
    cosr = np.cos(ang)[:, p].T.astype(BF)  # [128, T]
    sinr = np.sin(ang)[:, p].T.astype(BF)
    cm = np.zeros((CPQ * 128, QT), dtype=BF)
    for j in range(CPQ):
        tk = np.arange(128)[:, None]
        tq = np.arange(QT)[None, :]
        cm[j * 128 : (j + 1) * 128] = (tq >= j * 128 + tk).astype(BF)

    in_maps = []
    for c in range(8):
        b, r = c // 4, c % 4
        qs = slice(r * VF, (r + 1) * VF)
        qr = qkv_w[0 * D :][qs] * n1_w[None, :]
        kr = qkv_w[1 * D :][qs] * n1_w[None, :]
        vr = qkv_w[2 * D :][qs] * n1_w[None, :]
        ms = slice(r * MID, (r + 1) * MID)
        in_maps.append(
            {
                "x": np.ascontiguousarray(x[b, :T], np.float32),
                "qkw_t": np.ascontiguousarray(
                    np.concatenate([qr, kr], 0).T.astype(BF)
                ),
                "vw_m": np.ascontiguousarray(vr.T.astype(BF)),
                "ow_m": np.ascontiguousarray(o_w[:, qs].T.astype(BF)),
                "w1w_t": np.ascontiguousarray(
                    (w1_w[ms] * n2_w[None, :]).T.astype(BF)
                ),
                "wgw_t": np.ascontiguousarray(
                    (wg_w[ms] * n2_w[None, :]).T.astype(BF)
                ),
                "w2w_m": np.ascontiguousarray(w2_w[:, ms].T.astype(BF)),
                "cosr": cosr,
                "sinr": sinr,
                "cmask": cm,
                "ident": np.eye(128, dtype=BF),
            }
        )
    return in_maps


_CACHE = {}


def _get_nc(T):
    if T not in _CACHE:
        _CACHE[T] = build_nc(T, use_silu=True)
    return _CACHE[T]


def run(inputs, T=2048, trace=False):
    nc = _get_nc(T)
    in_maps = make_in_maps(T=T, **inputs)
    res = run_bass_kernel_spmd(nc, in_maps, core_ids=list(range(8)), trace=trace)
    out = np.stack([res.results[0]["out"], res.results[4]["out"]])
    return out, res


def kernel(**inputs):
    out, _ = run(inputs, T=2048)
    return out
